# revision 22
# baseline (speedup 1.0000x reference)
"""Trainium2 Bass kernel for GNN multi-head edge-softmax attention.

Problem: N=100000 nodes, E=1000000 edges, H=8 heads, d=8 per head.
  query/key per (edge,head): 8-dim dot -> logits [E,H]
  segment softmax over incoming edges of each dst node
  attention-weighted segment-sum of per-edge values -> [N,16,3], [N,16,1]

Strategy (edge parallelism, graph partitioned by destination node):
  * Host sorts edges by dst, partitions nodes into 8 contiguous ranges with
    ~equal edge counts; core i gets all edges of its node range.
  * Edges are packed into chunks of <=128 edges whose dst values span <=32
    consecutive nodes ("window").  Per chunk the device computes, via one
    one-hot matmul, the per-node partial sums of [exp(logit) | exp(logit)*v]
    (72 columns) over the chunk's 32-node window -> staging DRAM.
  * Softmax max-subtraction is skipped (logits are O(1), exp is safe in f32,
    softmax is shift-invariant so the result is identical up to rounding).
  * Host combines window rows per node (np.add.reduceat over sorted slots)
    and divides by the exp-sum -> final output.  This is the unshard step.
"""

import math
import os
import sys

import numpy as np

sys.path.insert(0, "/opt/trn_rl_repo")

N_NODES = 100000
N_EDGES = 1000000
HEADS = 8
DHEAD = 8          # 6 vec dims + 2 scalar dims per head
DCOL = HEADS * DHEAD  # 64 combined feature columns
W = 32             # node window per chunk
PCHUNK = 128       # edges per chunk
G = 32             # chunks per DMA group
PAYW = DHEAD + 1   # 9 payload cols per head... (not used; payload is 8+64)
NCORES = 8
INV_SQRT_D = 1.0 / math.sqrt(8.0)

# dtype knobs: stream edge tensors / staging output in bf16 to halve traffic
BF16_KQ = os.environ.get("BF16_KQ", "1") == "1"
BF16_V = os.environ.get("BF16_V", "1") == "1"
BF16_OUT = os.environ.get("BF16_OUT", "1") == "1"

_compiled = {}
_last = {}


def profile_last(trace_dir=None):
    """Re-run the last kernel invocation with NTFF profiling; returns exec ns."""
    from concourse.bass_utils import run_bass_kernel_spmd

    if "nc" not in _last:
        return None
    res = run_bass_kernel_spmd(
        _last["nc"],
        _last["in_maps"],
        core_ids=list(range(NCORES)),
        trace=True,
        tmpdir=trace_dir,
    )
    _last["profile"] = res
    return res.exec_time_ns


# --------------------------------------------------------------------------
# Bass kernel builder
# --------------------------------------------------------------------------
def _build_kernel(ng: int):
    """Build the SPMD Bass graph for `ng` groups of G chunks of 128 edges."""
    import concourse.bacc as bacc
    import concourse.mybir as mybir
    from concourse.tile import TileContext

    f32 = mybir.dt.float32
    bf16 = mybir.dt.bfloat16
    kq_t = bf16 if BF16_KQ else f32
    v_t = bf16 if BF16_V else f32
    out_t = bf16 if BF16_OUT else f32
    nc = bacc.Bacc("TRN2", target_bir_lowering=False, debug=False)

    k_d = nc.dram_tensor("k", [ng, 128, G * DCOL], kq_t, kind="ExternalInput")
    lid_d = nc.dram_tensor("lid", [ng, 128, G], bf16, kind="ExternalInput")
    q_d = nc.dram_tensor("q", [ng, 128, G * DCOL], kq_t, kind="ExternalInput")
    v_d = nc.dram_tensor("v", [ng, 128, G * DCOL], v_t, kind="ExternalInput")
    iota_d = nc.dram_tensor("iota", [128, G, W], bf16, kind="ExternalInput")
    out_d = nc.dram_tensor("out", [ng, W, G * 72], out_t, kind="ExternalOutput")

    with TileContext(nc) as tc:
        with (
            tc.tile_pool(name="const", bufs=1) as cpool,
            tc.tile_pool(name="io", bufs=4) as io,
            tc.tile_pool(name="work", bufs=3) as wk,
            tc.tile_pool(name="outp", bufs=3) as op,
            tc.tile_pool(name="psum", bufs=8, space="PSUM") as pp,
        ):
            iota_t = cpool.tile([128, G * W], bf16)
            nc.sync.dma_start(out=iota_t[:], in_=iota_d[:, :, :])

            for gi in range(ng):
                kt = io.tile([128, G * DCOL], kq_t, tag="kt")
                qt = io.tile([128, G * DCOL], kq_t, tag="qt")
                vt = io.tile([128, G * DCOL], v_t, tag="vt")
                ltile = io.tile([128, G], bf16, tag="lt")
                nc.sync.dma_start(out=kt[:], in_=k_d[gi, :, :])
                nc.scalar.dma_start(out=qt[:], in_=q_d[gi, :, :])
                nc.sync.dma_start(out=vt[:], in_=v_d[gi, :, :])
                nc.scalar.dma_start(out=ltile[:], in_=lid_d[gi, :, :])
                lt = ltile[:, :, None]

                # per-edge elementwise k*q on GPSIMD (DVE is the bottleneck),
                # then per-head dot via grouped reduce on DVE
                prod = wk.tile([128, G * DCOL], bf16, tag="prod")
                nc.gpsimd.tensor_tensor(
                    out=prod[:], in0=kt[:], in1=qt[:], op=mybir.AluOpType.mult
                )
                logits = wk.tile([128, G * HEADS], f32, tag="logits")
                nc.vector.tensor_reduce(
                    out=logits[:],
                    in_=prod[:].rearrange("p (gh d) -> p gh d", d=DHEAD),
                    axis=mybir.AxisListType.X,
                    op=mybir.AluOpType.add,
                )
                # ex = exp(logits / sqrt(d))  (max-subtraction skipped; safe in f32)
                ex = wk.tile([128, G * HEADS], bf16, tag="ex")
                nc.scalar.activation(
                    out=ex[:],
                    in_=logits[:],
                    func=mybir.ActivationFunctionType.Exp,
                    scale=INV_SQRT_D,
                )
                # one-hot selection matrix: oh[e, g*W + j] = (lid[e,g] == j)
                oh = wk.tile([128, G * W], bf16, tag="oh")
                nc.vector.tensor_tensor(
                    out=oh[:].rearrange("p (g w) -> p g w", w=W),
                    in0=lt.to_broadcast([128, G, W]),
                    in1=iota_t[:].rearrange("p (g w) -> p g w", w=W),
                    op=mybir.AluOpType.is_equal,
                )
                # wv = v * ex (broadcast per head), contiguous tile
                wv = wk.tile([128, G * DCOL], bf16, tag="wv")
                nc.vector.tensor_tensor(
                    out=wv[:].rearrange("p (g h d) -> p g h d", h=HEADS, d=DHEAD),
                    in0=vt[:].rearrange("p (g h d) -> p g h d", h=HEADS, d=DHEAD),
                    in1=ex[:]
                    .rearrange("p (g h) -> p g h", h=HEADS)[:, :, :, None]
                    .to_broadcast([128, G, HEADS, DHEAD]),
                    op=mybir.AluOpType.mult,
                )
                # scatter: psum[j, 0:8]=sum_e oh*ex ; psum[j, 8:72]=sum_e oh*wv
                ot = op.tile([W, G * 72], out_t, tag="ot")
                for g4 in range(G // 4):
                    ps = pp.tile([W, 4 * 72], f32, tag="ps")
                    for j in range(4):
                        g = g4 * 4 + j
                        nc.tensor.matmul(
                            out=ps[:, j * 72 : j * 72 + HEADS],
                            lhsT=oh[:, g * W : (g + 1) * W],
                            rhs=ex[:, g * HEADS : (g + 1) * HEADS],
                            start=True,
                            stop=True,
                        )
                        nc.tensor.matmul(
                            out=ps[:, j * 72 + HEADS : (j + 1) * 72],
                            lhsT=oh[:, g * W : (g + 1) * W],
                            rhs=wv[:, g * DCOL : (g + 1) * DCOL],
                            start=True,
                            stop=True,
                        )
                    nc.scalar.copy(
                        out=ot[:, g4 * 4 * 72 : (g4 + 1) * 4 * 72], in_=ps[:]
                    )
                nc.scalar.dma_start(out=out_d[gi, :, :], in_=ot[:])
    nc.compile()
    return nc


# --------------------------------------------------------------------------
# Host-side preprocessing / postprocessing
# --------------------------------------------------------------------------
def _combine_features(vec, scalar):
    """[B,16,3]+[B,16,1] -> [B, 64] with per-head layout [6 vec | 2 scalar]."""
    B = vec.shape[0]
    v = np.ascontiguousarray(vec, dtype=np.float32).reshape(B, HEADS, 6)
    s = np.ascontiguousarray(scalar, dtype=np.float32).reshape(B, HEADS, 2)
    return np.concatenate([v, s], axis=-1).reshape(B, DCOL)


def _chunk_edges(dst_sorted):
    """Greedy chunking of sorted dst values.

    Returns (chunk_id, pos_in_chunk, base_node, lid) per edge plus per-chunk
    arrays (bases, lid_max).  Each chunk has <=128 edges spanning <W nodes.
    """
    n = dst_sorted.shape[0]
    chunk_id = np.empty(n, np.int64)
    pos = np.empty(n, np.int64)
    lid = np.empty(n, np.int64)
    bases = []
    lid_max = []
    i = 0
    c = 0
    while i < n:
        b = dst_sorted[i]
        # edges fitting the node window [b, b+W)
        hi = np.searchsorted(dst_sorted, b + W, side="left")
        j = min(i + PCHUNK, hi, n)
        cnt = j - i
        chunk_id[i:j] = c
        pos[i:j] = np.arange(cnt)
        lid[i:j] = dst_sorted[i:j] - b
        bases.append(b)
        lid_max.append(int(dst_sorted[j - 1] - b))
        i = j
        c += 1
    return chunk_id, pos, lid, np.asarray(bases), np.asarray(lid_max)


def _prep(q_vec, q_scalar, k_vec, k_scalar, v_vec, v_scalar, dst):
    E = k_vec.shape[0]
    N = q_vec.shape[0]
    dst = np.asarray(dst).astype(np.int64)

    query = _combine_features(q_vec, q_scalar)   # [N, 64]
    key = _combine_features(k_vec, k_scalar)     # [E, 64]
    val = _combine_features(v_vec, v_scalar)     # [E, 64]

    perm = np.argsort(dst, kind="stable")
    dst_s = dst[perm]

    # split edges evenly across cores, snapped to node boundaries
    bounds = [0]
    for i in range(1, NCORES):
        t = i * E // NCORES
        # move t forward so a node's edges never straddle cores
        while t < E and t > 0 and dst_s[t] == dst_s[t - 1]:
            t += 1
        bounds.append(t)
    bounds.append(E)

    per_core = []
    for i in range(NCORES):
        e0, e1 = bounds[i], bounds[i + 1]
        cid, pos, lid, bases, lid_max = _chunk_edges(dst_s[e0:e1])
        per_core.append((e0, e1, cid, pos, lid, bases, lid_max))

    c_max = max(len(pc[5]) for pc in per_core)
    ng = (c_max + G - 1) // G
    c_tot = ng * G

    import ml_dtypes as _mld

    iota = np.broadcast_to(
        np.arange(W, dtype=np.float32), (128, G, W)
    ).astype(_mld.bfloat16)

    import ml_dtypes

    kq_np = ml_dtypes.bfloat16 if BF16_KQ else np.float32
    v_np = ml_dtypes.bfloat16 if BF16_V else np.float32

    in_maps = []
    maps = []
    for i in range(NCORES):
        e0, e1, cid, pos, lid, bases, lid_max = per_core[i]
        nreal = len(bases)
        flat_idx = cid * PCHUNK + pos  # destination slot per edge

        def pack(arr_sorted, dt, lid_col=None):
            ncol = DCOL + (1 if lid_col is not None else 0)
            flat = np.zeros((c_tot * PCHUNK, ncol), dt)
            flat[flat_idx, :DCOL] = arr_sorted[e0:e1]
            if lid_col is not None:
                flat[:, DCOL] = lid_col
            # [C,128,ncol] -> [NG, 128, G*ncol]
            return np.ascontiguousarray(
                flat.reshape(ng, G, PCHUNK, ncol)
                .transpose(0, 2, 1, 3)
                .reshape(ng, PCHUNK, G * ncol)
            )

        lid_flat = np.full(c_tot * PCHUNK, -1.0, np.float32)
        lid_flat[flat_idx] = lid.astype(np.float32)

        lid_dev = np.ascontiguousarray(
            lid_flat.reshape(ng, G, PCHUNK).transpose(0, 2, 1)
        ).astype(_mld.bfloat16)

        in_maps.append(
            {
                "k": pack(key[perm], kq_np),
                "q": pack(query[dst_s], kq_np),
                "v": pack(val[perm], v_np),
                "lid": lid_dev,
                "iota": iota,
            }
        )
        # host combine map: rows (chunk c, slot j<=lid_max[c]) -> node bases[c]+j
        rows = []
        nodes = []
        for c in range(nreal):
            m = lid_max[c] + 1
            rows.append(c * W + np.arange(m))
            nodes.append(bases[c] + np.arange(m))
        maps.append(
            (np.concatenate(rows) if rows else np.zeros(0, np.int64),
             np.concatenate(nodes) if nodes else np.zeros(0, np.int64))
        )
    return in_maps, maps, ng, N


def _combine(stages, maps, ng, N):
    """stages: per-core staging arrays [NG, W, G*72] -> final output tuple."""
    c_tot = ng * G
    all_rows = []
    all_nodes = []
    for i in range(NCORES):
        stg = np.asarray(stages[i]).astype(np.float32)
        stg = np.ascontiguousarray(
            stg.reshape(ng, W, G, 72).transpose(0, 2, 1, 3)
        ).reshape(c_tot * W, 72)
        rows_idx, nodes = maps[i]
        all_rows.append(stg[rows_idx])
        all_nodes.append(nodes)
    rows = np.concatenate(all_rows, axis=0)
    nodes = np.concatenate(all_nodes, axis=0)
    # nodes is non-decreasing; combine equal-node runs
    starts = np.flatnonzero(np.r_[True, np.diff(nodes) > 0])
    sums = np.add.reduceat(rows, starts, axis=0)  # [K, 72]
    uniq = nodes[starts]
    s = sums[:, :HEADS]                          # [K, 8]
    U = sums[:, HEADS:].reshape(-1, HEADS, DHEAD)  # [K, 8, 8]
    outc = np.zeros((N, HEADS, DHEAD), np.float32)
    valid = s[:, 0] > 0  # zero-degree nodes inside a window stay 0
    outc[uniq[valid]] = U[valid] / s[valid][:, :, None]
    out_vec = np.ascontiguousarray(outc[:, :, :6]).reshape(N, 16, 3)
    out_scalar = np.ascontiguousarray(outc[:, :, 6:]).reshape(N, 16, 1)
    return out_vec, out_scalar


def kernel(q_vec, q_scalar, k_vec, k_scalar, v_vec, v_scalar, dst):
    from concourse.bass_utils import run_bass_kernel_spmd

    N = q_vec.shape[0]
    in_maps, maps, ng, N = _prep(
        q_vec, q_scalar, k_vec, k_scalar, v_vec, v_scalar, dst
    )
    key_ = ("k", ng)
    if key_ not in _compiled:
        print(f"[kernel] building bass graph ng={ng}", flush=True)
        _compiled[key_] = _build_kernel(ng)
    nc = _compiled[key_]

    print("[kernel] launching on 8 cores", flush=True)
    res = run_bass_kernel_spmd(nc, in_maps, core_ids=list(range(NCORES)))
    print("[kernel] run complete", flush=True)
    _last["nc"] = nc
    _last["in_maps"] = in_maps
    return _combine([res.results[i]["out"] for i in range(NCORES)], maps, ng, N)


# revision 23
# speedup vs baseline: 1.4528x; 1.4528x over previous
"""Trainium2 Bass kernel for GNN multi-head edge-softmax attention.

Problem: N=100000 nodes, E=1000000 edges, H=8 heads, d=8 per head.
  query/key per (edge,head): 8-dim dot -> logits [E,H]
  segment softmax over incoming edges of each dst node
  attention-weighted segment-sum of per-edge values -> [N,16,3], [N,16,1]

Strategy (edge parallelism, graph partitioned by destination node):
  * Host sorts edges by dst, partitions nodes into 8 contiguous ranges with
    ~equal edge counts; core i gets all edges of its node range.
  * Edges are packed into chunks of <=128 edges whose dst values span <=32
    consecutive nodes ("window").  Per chunk the device computes, via one
    one-hot matmul, the per-node partial sums of [exp(logit) | exp(logit)*v]
    (72 columns) over the chunk's 32-node window -> staging DRAM.
  * Softmax max-subtraction is skipped (logits are O(1), exp is safe in f32,
    softmax is shift-invariant so the result is identical up to rounding).
  * Host combines window rows per node (np.add.reduceat over sorted slots)
    and divides by the exp-sum -> final output.  This is the unshard step.
"""

import math
import os
import sys

import numpy as np

sys.path.insert(0, "/opt/trn_rl_repo")

N_NODES = 100000
N_EDGES = 1000000
HEADS = 8
DHEAD = 8          # 6 vec dims + 2 scalar dims per head
DCOL = HEADS * DHEAD  # 64 combined feature columns
W = 32             # node window per chunk
PCHUNK = 128       # edges per chunk
G = 32             # chunks per DMA group
PAYW = DHEAD + 1   # 9 payload cols per head... (not used; payload is 8+64)
NCORES = 8
INV_SQRT_D = 1.0 / math.sqrt(8.0)

# dtype knobs: stream edge tensors / staging output in bf16 to halve traffic
BF16_KQ = os.environ.get("BF16_KQ", "1") == "1"
BF16_V = os.environ.get("BF16_V", "1") == "1"
BF16_OUT = os.environ.get("BF16_OUT", "1") == "1"

_compiled = {}
_last = {}


def profile_last(trace_dir=None):
    """Re-run the last kernel invocation with NTFF profiling; returns exec ns."""
    from concourse.bass_utils import run_bass_kernel_spmd

    if "nc" not in _last:
        return None
    res = run_bass_kernel_spmd(
        _last["nc"],
        _last["in_maps"],
        core_ids=list(range(NCORES)),
        trace=True,
        tmpdir=trace_dir,
    )
    _last["profile"] = res
    return res.exec_time_ns


# --------------------------------------------------------------------------
# Bass kernel builder
# --------------------------------------------------------------------------
def _build_kernel(ng: int):
    """Build the SPMD Bass graph for `ng` groups of G chunks of 128 edges."""
    import concourse.bacc as bacc
    import concourse.mybir as mybir
    from concourse.tile import TileContext

    f32 = mybir.dt.float32
    bf16 = mybir.dt.bfloat16
    kq_t = bf16 if BF16_KQ else f32
    v_t = bf16 if BF16_V else f32
    out_t = bf16 if BF16_OUT else f32
    nc = bacc.Bacc("TRN2", target_bir_lowering=False, debug=False)

    k_d = nc.dram_tensor("k", [ng, 128, G * DCOL], kq_t, kind="ExternalInput")
    lid_d = nc.dram_tensor("lid", [ng, 128, G], bf16, kind="ExternalInput")
    q_d = nc.dram_tensor("q", [ng, 128, G * DCOL], kq_t, kind="ExternalInput")
    v_d = nc.dram_tensor("v", [ng, 128, G * DCOL], v_t, kind="ExternalInput")
    iota_d = nc.dram_tensor("iota", [128, G, W], bf16, kind="ExternalInput")
    out_d = nc.dram_tensor("out", [ng, W, G * 72], out_t, kind="ExternalOutput")

    with TileContext(nc) as tc:
        with (
            tc.tile_pool(name="const", bufs=1) as cpool,
            tc.tile_pool(name="io", bufs=4) as io,
            tc.tile_pool(name="work", bufs=3) as wk,
            tc.tile_pool(name="outp", bufs=3) as op,
            tc.tile_pool(name="psum", bufs=8, space="PSUM") as pp,
        ):
            iota_t = cpool.tile([128, G * W], bf16)
            nc.sync.dma_start(out=iota_t[:], in_=iota_d[:, :, :])

            for gi in range(ng):
                kt = io.tile([128, G * DCOL], kq_t, tag="kt")
                qt = io.tile([128, G * DCOL], kq_t, tag="qt")
                vt = io.tile([128, G * DCOL], v_t, tag="vt")
                ltile = io.tile([128, G], bf16, tag="lt")
                nc.sync.dma_start(out=kt[:], in_=k_d[gi, :, :])
                nc.scalar.dma_start(out=qt[:], in_=q_d[gi, :, :])
                nc.sync.dma_start(out=vt[:], in_=v_d[gi, :, :])
                nc.scalar.dma_start(out=ltile[:], in_=lid_d[gi, :, :])
                lt = ltile[:, :, None]

                # per-edge elementwise k*q, then per-head dot via grouped reduce
                prod = wk.tile([128, G * DCOL], bf16, tag="prod")
                nc.vector.tensor_tensor(
                    out=prod[:], in0=kt[:], in1=qt[:], op=mybir.AluOpType.mult
                )
                logits = wk.tile([128, G * HEADS], f32, tag="logits")
                nc.vector.tensor_reduce(
                    out=logits[:],
                    in_=prod[:].rearrange("p (gh d) -> p gh d", d=DHEAD),
                    axis=mybir.AxisListType.X,
                    op=mybir.AluOpType.add,
                )
                # ex = exp(logits / sqrt(d))  (max-subtraction skipped; safe in f32)
                ex = wk.tile([128, G * HEADS], bf16, tag="ex")
                nc.scalar.activation(
                    out=ex[:],
                    in_=logits[:],
                    func=mybir.ActivationFunctionType.Exp,
                    scale=INV_SQRT_D,
                )
                # one-hot selection matrix: oh[e, g*W + j] = (lid[e,g] == j)
                oh = wk.tile([128, G * W], bf16, tag="oh")
                nc.vector.tensor_tensor(
                    out=oh[:].rearrange("p (g w) -> p g w", w=W),
                    in0=lt.to_broadcast([128, G, W]),
                    in1=iota_t[:].rearrange("p (g w) -> p g w", w=W),
                    op=mybir.AluOpType.is_equal,
                )
                # wv = v * ex (broadcast per head), contiguous tile
                wv = wk.tile([128, G * DCOL], bf16, tag="wv")
                nc.vector.tensor_tensor(
                    out=wv[:].rearrange("p (g h d) -> p g h d", h=HEADS, d=DHEAD),
                    in0=vt[:].rearrange("p (g h d) -> p g h d", h=HEADS, d=DHEAD),
                    in1=ex[:]
                    .rearrange("p (g h) -> p g h", h=HEADS)[:, :, :, None]
                    .to_broadcast([128, G, HEADS, DHEAD]),
                    op=mybir.AluOpType.mult,
                )
                # scatter: psum[j, 0:8]=sum_e oh*ex ; psum[j, 8:72]=sum_e oh*wv
                ot = op.tile([W, G * 72], out_t, tag="ot")
                for g4 in range(G // 4):
                    ps = pp.tile([W, 4 * 72], f32, tag="ps")
                    for j in range(4):
                        g = g4 * 4 + j
                        nc.tensor.matmul(
                            out=ps[:, j * 72 : j * 72 + HEADS],
                            lhsT=oh[:, g * W : (g + 1) * W],
                            rhs=ex[:, g * HEADS : (g + 1) * HEADS],
                            start=True,
                            stop=True,
                        )
                        nc.tensor.matmul(
                            out=ps[:, j * 72 + HEADS : (j + 1) * 72],
                            lhsT=oh[:, g * W : (g + 1) * W],
                            rhs=wv[:, g * DCOL : (g + 1) * DCOL],
                            start=True,
                            stop=True,
                        )
                    nc.scalar.copy(
                        out=ot[:, g4 * 4 * 72 : (g4 + 1) * 4 * 72], in_=ps[:]
                    )
                nc.scalar.dma_start(out=out_d[gi, :, :], in_=ot[:])
    nc.compile()
    return nc


# --------------------------------------------------------------------------
# Host-side preprocessing / postprocessing
# --------------------------------------------------------------------------
def _combine_features(vec, scalar):
    """[B,16,3]+[B,16,1] -> [B, 64] with per-head layout [6 vec | 2 scalar]."""
    B = vec.shape[0]
    v = np.ascontiguousarray(vec, dtype=np.float32).reshape(B, HEADS, 6)
    s = np.ascontiguousarray(scalar, dtype=np.float32).reshape(B, HEADS, 2)
    return np.concatenate([v, s], axis=-1).reshape(B, DCOL)


def _chunk_edges(dst_sorted):
    """Greedy chunking of sorted dst values.

    Returns (chunk_id, pos_in_chunk, base_node, lid) per edge plus per-chunk
    arrays (bases, lid_max).  Each chunk has <=128 edges spanning <W nodes.
    """
    n = dst_sorted.shape[0]
    chunk_id = np.empty(n, np.int64)
    pos = np.empty(n, np.int64)
    lid = np.empty(n, np.int64)
    bases = []
    lid_max = []
    i = 0
    c = 0
    while i < n:
        b = dst_sorted[i]
        # edges fitting the node window [b, b+W)
        hi = np.searchsorted(dst_sorted, b + W, side="left")
        j = min(i + PCHUNK, hi, n)
        cnt = j - i
        chunk_id[i:j] = c
        pos[i:j] = np.arange(cnt)
        lid[i:j] = dst_sorted[i:j] - b
        bases.append(b)
        lid_max.append(int(dst_sorted[j - 1] - b))
        i = j
        c += 1
    return chunk_id, pos, lid, np.asarray(bases), np.asarray(lid_max)


def _prep(q_vec, q_scalar, k_vec, k_scalar, v_vec, v_scalar, dst):
    E = k_vec.shape[0]
    N = q_vec.shape[0]
    dst = np.asarray(dst).astype(np.int64)

    query = _combine_features(q_vec, q_scalar)   # [N, 64]
    key = _combine_features(k_vec, k_scalar)     # [E, 64]
    val = _combine_features(v_vec, v_scalar)     # [E, 64]

    perm = np.argsort(dst, kind="stable")
    dst_s = dst[perm]

    # split edges evenly across cores, snapped to node boundaries
    bounds = [0]
    for i in range(1, NCORES):
        t = i * E // NCORES
        # move t forward so a node's edges never straddle cores
        while t < E and t > 0 and dst_s[t] == dst_s[t - 1]:
            t += 1
        bounds.append(t)
    bounds.append(E)

    per_core = []
    for i in range(NCORES):
        e0, e1 = bounds[i], bounds[i + 1]
        cid, pos, lid, bases, lid_max = _chunk_edges(dst_s[e0:e1])
        per_core.append((e0, e1, cid, pos, lid, bases, lid_max))

    c_max = max(len(pc[5]) for pc in per_core)
    ng = (c_max + G - 1) // G
    c_tot = ng * G

    import ml_dtypes as _mld

    iota = np.broadcast_to(
        np.arange(W, dtype=np.float32), (128, G, W)
    ).astype(_mld.bfloat16)

    import ml_dtypes

    kq_np = ml_dtypes.bfloat16 if BF16_KQ else np.float32
    v_np = ml_dtypes.bfloat16 if BF16_V else np.float32

    in_maps = []
    maps = []
    for i in range(NCORES):
        e0, e1, cid, pos, lid, bases, lid_max = per_core[i]
        nreal = len(bases)
        flat_idx = cid * PCHUNK + pos  # destination slot per edge

        def pack(arr_sorted, dt, lid_col=None):
            ncol = DCOL + (1 if lid_col is not None else 0)
            flat = np.zeros((c_tot * PCHUNK, ncol), dt)
            flat[flat_idx, :DCOL] = arr_sorted[e0:e1]
            if lid_col is not None:
                flat[:, DCOL] = lid_col
            # [C,128,ncol] -> [NG, 128, G*ncol]
            return np.ascontiguousarray(
                flat.reshape(ng, G, PCHUNK, ncol)
                .transpose(0, 2, 1, 3)
                .reshape(ng, PCHUNK, G * ncol)
            )

        lid_flat = np.full(c_tot * PCHUNK, -1.0, np.float32)
        lid_flat[flat_idx] = lid.astype(np.float32)

        lid_dev = np.ascontiguousarray(
            lid_flat.reshape(ng, G, PCHUNK).transpose(0, 2, 1)
        ).astype(_mld.bfloat16)

        in_maps.append(
            {
                "k": pack(key[perm], kq_np),
                "q": pack(query[dst_s], kq_np),
                "v": pack(val[perm], v_np),
                "lid": lid_dev,
                "iota": iota,
            }
        )
        # host combine map: rows (chunk c, slot j<=lid_max[c]) -> node bases[c]+j
        rows = []
        nodes = []
        for c in range(nreal):
            m = lid_max[c] + 1
            rows.append(c * W + np.arange(m))
            nodes.append(bases[c] + np.arange(m))
        maps.append(
            (np.concatenate(rows) if rows else np.zeros(0, np.int64),
             np.concatenate(nodes) if nodes else np.zeros(0, np.int64))
        )
    return in_maps, maps, ng, N


def _combine(stages, maps, ng, N):
    """stages: per-core staging arrays [NG, W, G*72] -> final output tuple."""
    c_tot = ng * G
    all_rows = []
    all_nodes = []
    for i in range(NCORES):
        stg = np.asarray(stages[i]).astype(np.float32)
        stg = np.ascontiguousarray(
            stg.reshape(ng, W, G, 72).transpose(0, 2, 1, 3)
        ).reshape(c_tot * W, 72)
        rows_idx, nodes = maps[i]
        all_rows.append(stg[rows_idx])
        all_nodes.append(nodes)
    rows = np.concatenate(all_rows, axis=0)
    nodes = np.concatenate(all_nodes, axis=0)
    # nodes is non-decreasing; combine equal-node runs
    starts = np.flatnonzero(np.r_[True, np.diff(nodes) > 0])
    sums = np.add.reduceat(rows, starts, axis=0)  # [K, 72]
    uniq = nodes[starts]
    s = sums[:, :HEADS]                          # [K, 8]
    U = sums[:, HEADS:].reshape(-1, HEADS, DHEAD)  # [K, 8, 8]
    outc = np.zeros((N, HEADS, DHEAD), np.float32)
    valid = s[:, 0] > 0  # zero-degree nodes inside a window stay 0
    outc[uniq[valid]] = U[valid] / s[valid][:, :, None]
    out_vec = np.ascontiguousarray(outc[:, :, :6]).reshape(N, 16, 3)
    out_scalar = np.ascontiguousarray(outc[:, :, 6:]).reshape(N, 16, 1)
    return out_vec, out_scalar


def kernel(q_vec, q_scalar, k_vec, k_scalar, v_vec, v_scalar, dst):
    from concourse.bass_utils import run_bass_kernel_spmd

    N = q_vec.shape[0]
    in_maps, maps, ng, N = _prep(
        q_vec, q_scalar, k_vec, k_scalar, v_vec, v_scalar, dst
    )
    key_ = ("k", ng)
    if key_ not in _compiled:
        print(f"[kernel] building bass graph ng={ng}", flush=True)
        _compiled[key_] = _build_kernel(ng)
    nc = _compiled[key_]

    print("[kernel] launching on 8 cores", flush=True)
    res = run_bass_kernel_spmd(nc, in_maps, core_ids=list(range(NCORES)))
    print("[kernel] run complete", flush=True)
    _last["nc"] = nc
    _last["in_maps"] = in_maps
    return _combine([res.results[i]["out"] for i in range(NCORES)], maps, ng, N)


# revision 24
# speedup vs baseline: 1.5292x; 1.0526x over previous
"""Trainium2 Bass kernel for GNN multi-head edge-softmax attention.

Problem: N=100000 nodes, E=1000000 edges, H=8 heads, d=8 per head.
  query/key per (edge,head): 8-dim dot -> logits [E,H]
  segment softmax over incoming edges of each dst node
  attention-weighted segment-sum of per-edge values -> [N,16,3], [N,16,1]

Strategy (edge parallelism, graph partitioned by destination node):
  * Host sorts edges by dst, partitions nodes into 8 contiguous ranges with
    ~equal edge counts; core i gets all edges of its node range.
  * Edges are packed into chunks of <=128 edges whose dst values span <=32
    consecutive nodes ("window").  Per chunk the device computes, via one
    one-hot matmul, the per-node partial sums of [exp(logit) | exp(logit)*v]
    (72 columns) over the chunk's 32-node window -> staging DRAM.
  * Softmax max-subtraction is skipped (logits are O(1), exp is safe in f32,
    softmax is shift-invariant so the result is identical up to rounding).
  * Host combines window rows per node (np.add.reduceat over sorted slots)
    and divides by the exp-sum -> final output.  This is the unshard step.
"""

import math
import os
import sys

import numpy as np

sys.path.insert(0, "/opt/trn_rl_repo")

N_NODES = 100000
N_EDGES = 1000000
HEADS = 8
DHEAD = 8          # 6 vec dims + 2 scalar dims per head
DCOL = HEADS * DHEAD  # 64 combined feature columns
W = 20             # node window per chunk
PCHUNK = 128       # edges per chunk
G = 32             # chunks per DMA group
PAYW = DHEAD + 1   # 9 payload cols per head... (not used; payload is 8+64)
NCORES = 8
INV_SQRT_D = 1.0 / math.sqrt(8.0)

# dtype knobs: stream edge tensors / staging output in bf16 to halve traffic
BF16_KQ = os.environ.get("BF16_KQ", "1") == "1"
BF16_V = os.environ.get("BF16_V", "1") == "1"
BF16_OUT = os.environ.get("BF16_OUT", "1") == "1"

_compiled = {}
_last = {}


def profile_last(trace_dir=None):
    """Re-run the last kernel invocation with NTFF profiling; returns exec ns."""
    from concourse.bass_utils import run_bass_kernel_spmd

    if "nc" not in _last:
        return None
    res = run_bass_kernel_spmd(
        _last["nc"],
        _last["in_maps"],
        core_ids=list(range(NCORES)),
        trace=True,
        tmpdir=trace_dir,
    )
    _last["profile"] = res
    return res.exec_time_ns


# --------------------------------------------------------------------------
# Bass kernel builder
# --------------------------------------------------------------------------
def _build_kernel(ng: int):
    """Build the SPMD Bass graph for `ng` groups of G chunks of 128 edges."""
    import concourse.bacc as bacc
    import concourse.mybir as mybir
    from concourse.tile import TileContext

    f32 = mybir.dt.float32
    bf16 = mybir.dt.bfloat16
    kq_t = bf16 if BF16_KQ else f32
    v_t = bf16 if BF16_V else f32
    out_t = bf16 if BF16_OUT else f32
    nc = bacc.Bacc("TRN2", target_bir_lowering=False, debug=False)

    k_d = nc.dram_tensor("k", [ng, 128, G * DCOL], kq_t, kind="ExternalInput")
    lid_d = nc.dram_tensor("lid", [ng, 128, G], bf16, kind="ExternalInput")
    q_d = nc.dram_tensor("q", [ng, 128, G * DCOL], kq_t, kind="ExternalInput")
    v_d = nc.dram_tensor("v", [ng, 128, G * DCOL], v_t, kind="ExternalInput")
    iota_d = nc.dram_tensor("iota", [128, G, W], bf16, kind="ExternalInput")
    out_d = nc.dram_tensor("out", [ng, W, G * 72], out_t, kind="ExternalOutput")

    with TileContext(nc) as tc:
        with (
            tc.tile_pool(name="const", bufs=1) as cpool,
            tc.tile_pool(name="io", bufs=6) as io,
            tc.tile_pool(name="work", bufs=5) as wk,
            tc.tile_pool(name="outp", bufs=4) as op,
            tc.tile_pool(name="psum", bufs=8, space="PSUM") as pp,
        ):
            iota_t = cpool.tile([128, G * W], bf16)
            nc.sync.dma_start(out=iota_t[:], in_=iota_d[:, :, :])

            for gi in range(ng):
                kt = io.tile([128, G * DCOL], kq_t, tag="kt")
                qt = io.tile([128, G * DCOL], kq_t, tag="qt")
                vt = io.tile([128, G * DCOL], v_t, tag="vt")
                ltile = io.tile([128, G], bf16, tag="lt")
                nc.sync.dma_start(out=kt[:], in_=k_d[gi, :, :])
                nc.scalar.dma_start(out=qt[:], in_=q_d[gi, :, :])
                nc.sync.dma_start(out=vt[:], in_=v_d[gi, :, :])
                nc.scalar.dma_start(out=ltile[:], in_=lid_d[gi, :, :])
                lt = ltile[:, :, None]

                # per-edge elementwise k*q, then per-head dot via grouped reduce
                prod = wk.tile([128, G * DCOL], bf16, tag="prod")
                nc.vector.tensor_tensor(
                    out=prod[:], in0=kt[:], in1=qt[:], op=mybir.AluOpType.mult
                )
                logits = wk.tile([128, G * HEADS], f32, tag="logits")
                nc.vector.tensor_reduce(
                    out=logits[:],
                    in_=prod[:].rearrange("p (gh d) -> p gh d", d=DHEAD),
                    axis=mybir.AxisListType.X,
                    op=mybir.AluOpType.add,
                )
                # ex = exp(logits / sqrt(d))  (max-subtraction skipped; safe in f32)
                ex = wk.tile([128, G * HEADS], bf16, tag="ex")
                nc.scalar.activation(
                    out=ex[:],
                    in_=logits[:],
                    func=mybir.ActivationFunctionType.Exp,
                    scale=INV_SQRT_D,
                )
                # one-hot selection matrix: oh[e, g*W + j] = (lid[e,g] == j)
                oh = wk.tile([128, G * W], bf16, tag="oh")
                nc.vector.tensor_tensor(
                    out=oh[:].rearrange("p (g w) -> p g w", w=W),
                    in0=lt.to_broadcast([128, G, W]),
                    in1=iota_t[:].rearrange("p (g w) -> p g w", w=W),
                    op=mybir.AluOpType.is_equal,
                )
                # wv = v * ex (broadcast per head), contiguous tile
                wv = wk.tile([128, G * DCOL], bf16, tag="wv")
                nc.vector.tensor_tensor(
                    out=wv[:].rearrange("p (g h d) -> p g h d", h=HEADS, d=DHEAD),
                    in0=vt[:].rearrange("p (g h d) -> p g h d", h=HEADS, d=DHEAD),
                    in1=ex[:]
                    .rearrange("p (g h) -> p g h", h=HEADS)[:, :, :, None]
                    .to_broadcast([128, G, HEADS, DHEAD]),
                    op=mybir.AluOpType.mult,
                )
                # scatter: psum[j, 0:8]=sum_e oh*ex ; psum[j, 8:72]=sum_e oh*wv
                ot = op.tile([W, G * 72], out_t, tag="ot")
                for g4 in range(G // 4):
                    ps = pp.tile([W, 4 * 72], f32, tag="ps")
                    for j in range(4):
                        g = g4 * 4 + j
                        nc.tensor.matmul(
                            out=ps[:, j * 72 : j * 72 + HEADS],
                            lhsT=oh[:, g * W : (g + 1) * W],
                            rhs=ex[:, g * HEADS : (g + 1) * HEADS],
                            start=True,
                            stop=True,
                        )
                        nc.tensor.matmul(
                            out=ps[:, j * 72 + HEADS : (j + 1) * 72],
                            lhsT=oh[:, g * W : (g + 1) * W],
                            rhs=wv[:, g * DCOL : (g + 1) * DCOL],
                            start=True,
                            stop=True,
                        )
                    nc.scalar.copy(
                        out=ot[:, g4 * 4 * 72 : (g4 + 1) * 4 * 72], in_=ps[:]
                    )
                nc.scalar.dma_start(out=out_d[gi, :, :], in_=ot[:])
    nc.compile()
    return nc


# --------------------------------------------------------------------------
# Host-side preprocessing / postprocessing
# --------------------------------------------------------------------------
def _combine_features(vec, scalar):
    """[B,16,3]+[B,16,1] -> [B, 64] with per-head layout [6 vec | 2 scalar]."""
    B = vec.shape[0]
    v = np.ascontiguousarray(vec, dtype=np.float32).reshape(B, HEADS, 6)
    s = np.ascontiguousarray(scalar, dtype=np.float32).reshape(B, HEADS, 2)
    return np.concatenate([v, s], axis=-1).reshape(B, DCOL)


def _chunk_edges(dst_sorted):
    """Greedy chunking of sorted dst values.

    Returns (chunk_id, pos_in_chunk, base_node, lid) per edge plus per-chunk
    arrays (bases, lid_max).  Each chunk has <=128 edges spanning <W nodes.
    """
    n = dst_sorted.shape[0]
    chunk_id = np.empty(n, np.int64)
    pos = np.empty(n, np.int64)
    lid = np.empty(n, np.int64)
    bases = []
    lid_max = []
    i = 0
    c = 0
    while i < n:
        b = dst_sorted[i]
        # edges fitting the node window [b, b+W)
        hi = np.searchsorted(dst_sorted, b + W, side="left")
        j = min(i + PCHUNK, hi, n)
        cnt = j - i
        chunk_id[i:j] = c
        pos[i:j] = np.arange(cnt)
        lid[i:j] = dst_sorted[i:j] - b
        bases.append(b)
        lid_max.append(int(dst_sorted[j - 1] - b))
        i = j
        c += 1
    return chunk_id, pos, lid, np.asarray(bases), np.asarray(lid_max)


def _prep(q_vec, q_scalar, k_vec, k_scalar, v_vec, v_scalar, dst):
    E = k_vec.shape[0]
    N = q_vec.shape[0]
    dst = np.asarray(dst).astype(np.int64)

    query = _combine_features(q_vec, q_scalar)   # [N, 64]
    key = _combine_features(k_vec, k_scalar)     # [E, 64]
    val = _combine_features(v_vec, v_scalar)     # [E, 64]

    perm = np.argsort(dst, kind="stable")
    dst_s = dst[perm]

    # split edges evenly across cores, snapped to node boundaries
    bounds = [0]
    for i in range(1, NCORES):
        t = i * E // NCORES
        # move t forward so a node's edges never straddle cores
        while t < E and t > 0 and dst_s[t] == dst_s[t - 1]:
            t += 1
        bounds.append(t)
    bounds.append(E)

    per_core = []
    for i in range(NCORES):
        e0, e1 = bounds[i], bounds[i + 1]
        cid, pos, lid, bases, lid_max = _chunk_edges(dst_s[e0:e1])
        per_core.append((e0, e1, cid, pos, lid, bases, lid_max))

    c_max = max(len(pc[5]) for pc in per_core)
    ng = (c_max + G - 1) // G
    c_tot = ng * G

    import ml_dtypes as _mld

    iota = np.broadcast_to(
        np.arange(W, dtype=np.float32), (128, G, W)
    ).astype(_mld.bfloat16)

    import ml_dtypes

    kq_np = ml_dtypes.bfloat16 if BF16_KQ else np.float32
    v_np = ml_dtypes.bfloat16 if BF16_V else np.float32

    in_maps = []
    maps = []
    for i in range(NCORES):
        e0, e1, cid, pos, lid, bases, lid_max = per_core[i]
        nreal = len(bases)
        flat_idx = cid * PCHUNK + pos  # destination slot per edge

        def pack(arr_sorted, dt, lid_col=None):
            ncol = DCOL + (1 if lid_col is not None else 0)
            flat = np.zeros((c_tot * PCHUNK, ncol), dt)
            flat[flat_idx, :DCOL] = arr_sorted[e0:e1]
            if lid_col is not None:
                flat[:, DCOL] = lid_col
            # [C,128,ncol] -> [NG, 128, G*ncol]
            return np.ascontiguousarray(
                flat.reshape(ng, G, PCHUNK, ncol)
                .transpose(0, 2, 1, 3)
                .reshape(ng, PCHUNK, G * ncol)
            )

        lid_flat = np.full(c_tot * PCHUNK, -1.0, np.float32)
        lid_flat[flat_idx] = lid.astype(np.float32)

        lid_dev = np.ascontiguousarray(
            lid_flat.reshape(ng, G, PCHUNK).transpose(0, 2, 1)
        ).astype(_mld.bfloat16)

        in_maps.append(
            {
                "k": pack(key[perm], kq_np),
                "q": pack(query[dst_s], kq_np),
                "v": pack(val[perm], v_np),
                "lid": lid_dev,
                "iota": iota,
            }
        )
        # host combine map: rows (chunk c, slot j<=lid_max[c]) -> node bases[c]+j
        rows = []
        nodes = []
        for c in range(nreal):
            m = lid_max[c] + 1
            rows.append(c * W + np.arange(m))
            nodes.append(bases[c] + np.arange(m))
        maps.append(
            (np.concatenate(rows) if rows else np.zeros(0, np.int64),
             np.concatenate(nodes) if nodes else np.zeros(0, np.int64))
        )
    return in_maps, maps, ng, N


def _combine(stages, maps, ng, N):
    """stages: per-core staging arrays [NG, W, G*72] -> final output tuple."""
    c_tot = ng * G
    all_rows = []
    all_nodes = []
    for i in range(NCORES):
        stg = np.asarray(stages[i]).astype(np.float32)
        stg = np.ascontiguousarray(
            stg.reshape(ng, W, G, 72).transpose(0, 2, 1, 3)
        ).reshape(c_tot * W, 72)
        rows_idx, nodes = maps[i]
        all_rows.append(stg[rows_idx])
        all_nodes.append(nodes)
    rows = np.concatenate(all_rows, axis=0)
    nodes = np.concatenate(all_nodes, axis=0)
    # nodes is non-decreasing; combine equal-node runs
    starts = np.flatnonzero(np.r_[True, np.diff(nodes) > 0])
    sums = np.add.reduceat(rows, starts, axis=0)  # [K, 72]
    uniq = nodes[starts]
    s = sums[:, :HEADS]                          # [K, 8]
    U = sums[:, HEADS:].reshape(-1, HEADS, DHEAD)  # [K, 8, 8]
    outc = np.zeros((N, HEADS, DHEAD), np.float32)
    valid = s[:, 0] > 0  # zero-degree nodes inside a window stay 0
    outc[uniq[valid]] = U[valid] / s[valid][:, :, None]
    out_vec = np.ascontiguousarray(outc[:, :, :6]).reshape(N, 16, 3)
    out_scalar = np.ascontiguousarray(outc[:, :, 6:]).reshape(N, 16, 1)
    return out_vec, out_scalar


def kernel(q_vec, q_scalar, k_vec, k_scalar, v_vec, v_scalar, dst):
    from concourse.bass_utils import run_bass_kernel_spmd

    N = q_vec.shape[0]
    in_maps, maps, ng, N = _prep(
        q_vec, q_scalar, k_vec, k_scalar, v_vec, v_scalar, dst
    )
    key_ = ("k", ng)
    if key_ not in _compiled:
        print(f"[kernel] building bass graph ng={ng}", flush=True)
        _compiled[key_] = _build_kernel(ng)
    nc = _compiled[key_]

    print("[kernel] launching on 8 cores", flush=True)
    res = run_bass_kernel_spmd(nc, in_maps, core_ids=list(range(NCORES)))
    print("[kernel] run complete", flush=True)
    _last["nc"] = nc
    _last["in_maps"] = in_maps
    return _combine([res.results[i]["out"] for i in range(NCORES)], maps, ng, N)
